# revision 36
# baseline (speedup 1.0000x reference)
"""Trainium2 Bass kernel for nn_CreateGraphCondensation (retrieval_knn).

Reference semantics (see problem): for each point i with score<=THR, find the
K=64 nearest (squared L2) points j in the same row-split segment with
score>THR ("promoted"), sorted ascending by distance (ties: lower index
first). Promoted points get empty neighbour lists. Plus small ragged-index
outputs (rs_up / sel_idx_up) that are pure host bookkeeping.

Device strategy (8 NeuronCores, data-parallel over query rows):
  - Host compacts candidates (promoted points) per segment, queries
    (non-promoted points) are tiled 128 per partition-tile, each tile pure in
    one segment. Tiles are dealt round-robin to the 8 cores; every core runs
    an identical program (SPMD) on its own tile data.
  - Per tile, TensorEngine computes negd[i,j] = -(d2) = 2*q.c - |q|^2 - |c|^2
    via a single K=5 matmul: lhsT rows (qx,qy,qz,|q|^2,1),
    rhs rows (2cx,2cy,2cz,-1,-|c|^2). Pad candidates get -1e36.
  - VectorEngine extracts the top-64 (largest negd) per row with 8 rounds of
    max8 / max_index8 / match_replace8 — exact values, exact stable
    (lower-index-first) tie order, matching jax.lax.top_k.
  - Host maps window-local indices back to global ids, recomputes the exact
    distances/weights in fp32 (cheap O(N*K)) and assembles full outputs.
"""

import math

import numpy as np

K = 64
THR = 0.5
P = 128
NCORES = 8

_PROG_CACHE: dict = {}


def _build_program(T: int, W: int):
    """One SPMD NeuronCore program: T query-tiles of 128 rows, candidate
    window width W (multiple of 128)."""
    import concourse.bass as bass
    import concourse.mybir as mybir
    import concourse.tile as tile
    from concourse.tile_rust import add_dep_helper

    dt = mybir.dt
    nc = bass.Bass()
    S = P + W  # per-tile payload width: 128 query cols + W candidate cols
    G = (T + 3) // 4  # tiles packed 4-per-128-partitions at 32-row offsets
    qc = nc.dram_tensor("qc", [P, G * S], dt.float32, kind="ExternalInput")
    out = nc.dram_tensor("out", [P, 2 * T * K], dt.float32, kind="ExternalOutput")

    n_chunks = (W + 511) // 512

    with tile.TileContext(nc) as tc:
        with (
            tc.tile_pool(name="sb", bufs=2) as pool,
            tc.tile_pool(name="ps", bufs=8, space="PSUM") as psum,
        ):
            mega = pool.tile([P, G * S], dt.float32, tag="mega", bufs=1)
            dma_in = nc.sync.dma_start(mega, qc[:])
            mm_insts = []
            dve_insts = []
            act_insts = []
            outall = pool.tile([P, 2 * T * K], dt.float32, tag="outall", bufs=1)
            valsall = outall[:, : T * K]
            idxsall = outall[:, T * K :].bitcast(dt.uint32)
            for t in range(T):
                g, j = divmod(t, 4)
                base = g * S
                qs = mega[32 * j : 32 * j + 5, base : base + P]
                cs = mega[32 * j : 32 * j + 5, base + P : base + S]
                negd = pool.tile([P, W], dt.float32, tag="negd", bufs=T)
                for m in range(n_chunks):
                    w0 = m * 512
                    wlen = min(W, w0 + 512) - w0
                    ps_t = psum.tile([P, 512], dt.float32, tag="ps")
                    mm_insts.append(nc.tensor.matmul(
                        ps_t[:, :wlen], qs, cs[:, w0 : w0 + wlen],
                        start=True, stop=True, tile_position=(32 * j, 0),
                    ))
                    act_insts.append(
                        nc.scalar.copy(negd[:, w0 : w0 + wlen], ps_t[:, :wlen])
                    )
                vals = valsall[:, t * K : (t + 1) * K]
                idxs = idxsall[:, t * K : (t + 1) * K]
                for r in range(8):
                    sl = slice(8 * r, 8 * r + 8)
                    dve_insts.append(nc.vector.max(out=vals[:, sl], in_=negd))
                    dve_insts.append(nc.vector.max_index(
                        out=idxs[:, sl], in_max=vals[:, sl], in_values=negd
                    ))
                    if r < 7:
                        dve_insts.append(nc.vector.match_replace(
                            out=negd, in_to_replace=vals[:, sl],
                            in_values=negd, imm_value=-3.0e38,
                        ))
            dma_out = nc.sync.dma_start(out[:], outall)
            # Tail funnel: make the SP sequencer observe every proc's final
            # tick through single-wait NOPs, so the framework tail drain's
            # waits are all elided (this walrus build allows only one sync
            # wait command per instruction).
            for group in (mm_insts, dve_insts, act_insts, [dma_in], [dma_out]):
                nop = nc.sync.nop(nofuse=True, hint="tail_funnel")
                for bi in group:
                    add_dep_helper(nop.ins, bi.ins, True, "tail funnel")
    nc.finalize()
    return nc


def _get_program(T: int, W: int):
    key = (T, W)
    if key not in _PROG_CACHE:
        _PROG_CACHE[key] = _build_program(T, W)
    return _PROG_CACHE[key]


def _plan(score: np.ndarray, coords: np.ndarray, rs: np.ndarray):
    """Host-side planning: segments, candidate windows, query tiles."""
    N = coords.shape[0]
    promoted = score[:, 0] > THR
    seg = (np.searchsorted(rs, np.arange(N, dtype=rs.dtype), side="right") - 1).astype(
        np.int64
    )
    sq = np.sum(coords * coords, axis=-1, dtype=np.float32)

    seg_vals = np.unique(seg)
    cand_idx = {}  # seg value -> global indices of promoted points in it
    for u in seg_vals:
        cand_idx[u] = np.nonzero(promoted & (seg == u))[0]

    max_cand = max((len(v) for v in cand_idx.values()), default=0)
    W = max(P, int(math.ceil(max(max_cand, 1) / P)) * P)

    # candidate window payloads per segment: [5, W] matmul rhs + global map
    ct_win = {}
    win_map = {}
    for u in seg_vals:
        gi = cand_idx[u]
        n = len(gi)
        rhs = np.zeros((5, W), dtype=np.float32)
        rhs[4, :] = -1.0e36
        if n:
            c = coords[gi].astype(np.float32)
            rhs[0:3, :n] = (2.0 * c).T
            rhs[3, :n] = -1.0
            rhs[4, :n] = -sq[gi]
        m = np.full(W, -1, dtype=np.int64)
        m[:n] = gi
        ct_win[u] = rhs
        win_map[u] = m

    # query tiles: 128 rows each, single-segment
    tiles = []  # (seg value, np.ndarray of query global rows (<=128))
    for u in seg_vals:
        qi = np.nonzero((~promoted) & (seg == u))[0]
        for s in range(0, len(qi), P):
            tiles.append((u, qi[s : s + P]))

    T = max(1, int(math.ceil(len(tiles) / NCORES)))
    return promoted, sq, W, T, tiles, ct_win, win_map


LAST_RUN = {}


def _run_device(nc, in_maps):
    import os
    import time

    if os.environ.get("KNN_BASS_SIM") == "1":
        from concourse.bass_interp import CoreSim

        results = []
        for m in in_maps:
            sim = CoreSim(nc)
            for name, v in m.items():
                sim.tensor(name)[:] = v
            sim.simulate()
            results.append({"out": np.array(sim.tensor("out"))})
        return results
    from concourse.bass_utils import run_bass_kernel_spmd

    trace = os.environ.get("KNN_BASS_TRACE") == "1"
    t0 = time.perf_counter()
    try:
        r = run_bass_kernel_spmd(
            nc, in_maps, core_ids=list(range(NCORES)), trace=trace
        )
    except ModuleNotFoundError:
        r = run_bass_kernel_spmd(nc, in_maps, core_ids=list(range(NCORES)))
    LAST_RUN["wall_s"] = time.perf_counter() - t0
    LAST_RUN["exec_time_ns"] = r.exec_time_ns
    LAST_RUN["results_obj"] = r
    return r.results


def kernel(score, coords, rs):
    score = np.asarray(score, dtype=np.float32)
    coords = np.asarray(coords, dtype=np.float32)
    rs = np.asarray(rs, dtype=np.int32)
    N = coords.shape[0]

    promoted, sq, W, T, tiles, ct_win, win_map = _plan(score, coords, rs)

    # ---- small host-only outputs -------------------------------------
    rs_down = rs
    mask = promoted
    sel_idx_up = np.nonzero(mask)[0].astype(np.int32)[:, None]
    cum = np.cumsum(mask.astype(np.int32))
    rs_up = np.concatenate(
        [np.zeros((1,), np.int32), cum[rs[1:] - 1].astype(np.int32)]
    )

    # ---- build per-core device inputs --------------------------------
    S = P + W
    G = (T + 3) // 4
    in_maps = []
    core_tiles = []  # per core: list of (seg value, qrows) or None
    for c in range(NCORES):
        qct = np.zeros((P, G * S), dtype=np.float32)
        mine = []
        for t in range(T):
            g, j = divmod(t, 4)
            base = g * S
            block = np.zeros((5, S), dtype=np.float32)
            block[4, P:] = -1.0e36
            gidx = c + t * NCORES
            if gidx < len(tiles):
                u, qrows = tiles[gidx]
                n = len(qrows)
                qcd = coords[qrows].astype(np.float32)
                block[0:3, :n] = qcd.T
                block[3, :n] = sq[qrows]
                block[4, :n] = 1.0
                block[:, P:] = ct_win[u]
                mine.append((u, qrows))
            else:
                mine.append(None)
            qct[32 * j : 32 * j + 5, base : base + S] = block
        core_tiles.append(mine)
        in_maps.append({"qc": qct})

    # ---- run on the 8 NeuronCores ------------------------------------
    nc = _get_program(T, W)
    results = _run_device(nc, in_maps)

    # ---- host assembly -----------------------------------------------
    nidx = np.full((N, K), -1, dtype=np.int64)
    for c in range(NCORES):
        o = np.ascontiguousarray(results[c]["out"])  # [P, 2*T*K] f32
        oi = o[:, T * K :].view(np.uint32).astype(np.int64)  # window-local
        for t, info in enumerate(core_tiles[c]):
            if info is None:
                continue
            u, qrows = info
            n = len(qrows)
            nidx[qrows] = win_map[u][oi[:n, t * K : (t + 1) * K]]

    valid = nidx >= 0
    safe = np.where(valid, nidx, 0)
    # exact fp32 distance recompute, mirroring the reference expression
    qg = coords  # [N, 3]
    cg = coords[safe]  # [N, K, 3]
    dot = np.einsum("nd,nkd->nk", qg, cg, dtype=np.float32).astype(np.float32)
    d2 = (sq[:, None] + sq[safe]) - np.float32(2.0) * dot
    d2 = np.maximum(d2, np.float32(0.0))
    d2 = np.where(valid, d2, np.float32(0.0)).astype(np.float32)

    z = np.exp(-d2, dtype=np.float32)
    zm = z.max(axis=-1, keepdims=True)
    e = np.exp(z - zm, dtype=np.float32)
    weights = (e / e.sum(axis=-1, keepdims=True, dtype=np.float32)).astype(np.float32)

    nidx_down = nidx.astype(np.int32)
    return (rs_down, rs_up, nidx_down, d2, sel_idx_up, weights)
